# revision 4
# baseline (speedup 1.0000x reference)
"""DETR scene-graph predicate head on 8 Trainium2 NeuronCores.

Math: logits[l,b,r,:] = concat(hs[l,b,q_sub], hs[l,b,q_obj]) @ W_pred.T + b_pred
with q_sub[r] = src_indices[pos_sub[r]], pos_* derived from (tgt_perm
inverse, relationships) — pure integer index math, done on host.

Strategy (batch axis sharded 8 ways; L*B/8 = 192 (layer,image) blocks/core):
  - hs is staged in DRAM as bf16 rows [NB*101, 256]. A gpsimd dma_gather
    (transpose=True) pulls exactly the 64 matched rows per block
    (hs[src_indices[k]], 512B each) d-on-partitions:
    gt[p, c, j*64+k] = hs_row[blk_j*101+src[k]][c*128+p]. One gather per
    group of 12 blocks (768 descriptors — the SWDGE descriptor ring caps
    a single gather at <1024).
  - PE projects matched rows: proj[k,:] = [g_k.Ws | g_k.Wo] (2 accumulating
    N=102 matmuls per block, block pairs col-packed via tile_position
    (0,0)/(0,64)), psum -> SBUF bf16 cast on DVE/ACT.
  - Per-relation expansion stays on the PE: lg[r,p] = sum_k ohs[k,r]
    proj[k,p] + oho[k,r] proj[k,51+p] with the position one-hots shipped
    from host as u8 and cast to bf16. Even/odd blocks run in disjoint PE
    quadrants (0,0)/(64,64).
  - One bias add + bf16 store per group; host unpacks to [L, B, R, P] f32.

hs/W/proj/one-hot are bf16 on-chip (psum f32), logits stored bf16:
~6e-3 relative error vs the f32 reference.
"""

import sys

import numpy as np

L, B, Q1, D = 6, 256, 101, 256
M, R, P = 64, 64, 51
NCORES = 8
BLOC = B // NCORES          # images per core
NB = L * BLOC               # (layer, image) blocks per core
NBG = 12                    # blocks per gather/output group
NGRP = NB // NBG            # groups per core (16)
PPG = NBG // 2              # block pairs per group (6)
NIDX = NBG * M              # gather indices per group (768)
P2 = 2 * P                  # projected row width (102)

_CACHE = {}


def _build_program():
    import concourse.bacc as bacc
    import concourse.mybir as mybir
    import concourse.tile as tile
    from contextlib import ExitStack

    f32 = mybir.dt.float32
    bf16 = mybir.dt.bfloat16
    i16 = mybir.dt.int16
    u8 = mybir.dt.uint8
    nc = bacc.Bacc("TRN2", target_bir_lowering=False, debug=False)

    hs = nc.dram_tensor("hs", [NB * Q1, D], bf16, kind="ExternalInput").ap()
    midx = nc.dram_tensor("midx", [NGRP, 128, NIDX // 16], i16,
                          kind="ExternalInput").ap()
    ohp = nc.dram_tensor("ohp", [NGRP, 128, PPG * 2 * R], u8,
                         kind="ExternalInput").ap()
    wcat = nc.dram_tensor("wcat", [128, 2, P2], bf16,
                          kind="ExternalInput").ap()
    bias = nc.dram_tensor("bias", [128, PPG * P], f32,
                          kind="ExternalInput").ap()
    out = nc.dram_tensor("out", [NGRP, 128, PPG * P], bf16,
                         kind="ExternalOutput").ap()

    with tile.TileContext(nc) as tc, ExitStack() as ctx:
        const = ctx.enter_context(tc.tile_pool(name="const", bufs=1))
        inp = ctx.enter_context(tc.tile_pool(name="inp", bufs=3))
        gpool = ctx.enter_context(tc.tile_pool(name="gpool", bufs=3))
        ohb = ctx.enter_context(tc.tile_pool(name="ohb", bufs=3))
        prp = ctx.enter_context(tc.tile_pool(name="prp", bufs=4))
        outp = ctx.enter_context(tc.tile_pool(name="outp", bufs=3))
        psP = ctx.enter_context(tc.tile_pool(name="psP", bufs=4, space="PSUM"))
        psO = ctx.enter_context(tc.tile_pool(name="psO", bufs=3, space="PSUM"))

        wc_t = const.tile([128, 2, P2], bf16)
        nc.sync.dma_start(out=wc_t[:], in_=wcat[:])
        bias_t = const.tile([128, PPG * P], f32)
        nc.sync.dma_start(out=bias_t[:], in_=bias[:])

        for g in range(NGRP):
            idx_t = inp.tile([128, NIDX // 16], i16, tag="idx")
            nc.sync.dma_start(out=idx_t[:], in_=midx[g])
            gt = gpool.tile([128, 2, NIDX], bf16, tag="gt")
            nc.gpsimd.dma_gather(gt[:], hs, idx_t[:], NIDX, NIDX, D,
                                 elem_step=D, transpose=True)

            oh_u = inp.tile([128, PPG * 2 * R], u8, tag="ohu")
            nc.sync.dma_start(out=oh_u[:], in_=ohp[g])
            oh_b = ohb.tile([128, PPG * 2 * R], bf16, tag="ohb")
            if g % 2 == 0:
                nc.vector.tensor_copy(out=oh_b[:], in_=oh_u[:])
            else:
                nc.scalar.copy(out=oh_b[:], in_=oh_u[:])

            pO = psO.tile([128, PPG * P], f32, tag="pO")
            for half in range(2):            # pairs 0-2 / 3-5 share a bank
                pP = psP.tile([128, 3, P2], f32, tag="pP")
                for i in range(3):
                    pk = half * 3 + i
                    j0, j1 = 2 * pk, 2 * pk + 1
                    for c in range(2):
                        nc.tensor.matmul(
                            out=pP[0:M, i, :],
                            lhsT=gt[:, c, j0 * M:(j0 + 1) * M],
                            rhs=wc_t[:, c, :],
                            start=(c == 0), stop=(c == 1),
                            tile_position=(0, 0))
                        nc.tensor.matmul(
                            out=pP[M:2 * M, i, :],
                            lhsT=gt[:, c, j1 * M:(j1 + 1) * M],
                            rhs=wc_t[:, c, :],
                            start=(c == 0), stop=(c == 1),
                            tile_position=(0, 64))
                pr = prp.tile([128, 3, P2], bf16, tag="pr")
                if (g + half) % 2 == 0:
                    nc.vector.tensor_copy(out=pr[:], in_=pP[:])
                else:
                    nc.scalar.copy(out=pr[:], in_=pP[:])

                for i in range(3):
                    pk = half * 3 + i
                    o0 = pO[0:R, pk * P:(pk + 1) * P]
                    o1 = pO[R:2 * R, pk * P:(pk + 1) * P]
                    ohc = pk * 2 * R
                    for h in range(2):       # sub / obj halves accumulate
                        nc.tensor.matmul(
                            out=o0,
                            lhsT=oh_b[0:M, ohc + h * R:ohc + (h + 1) * R],
                            rhs=pr[0:M, i, h * P:(h + 1) * P],
                            start=(h == 0), stop=(h == 1),
                            tile_position=(0, 0))
                        nc.tensor.matmul(
                            out=o1,
                            lhsT=oh_b[M:2 * M, ohc + h * R:ohc + (h + 1) * R],
                            rhs=pr[M:2 * M, i, h * P:(h + 1) * P],
                            start=(h == 0), stop=(h == 1),
                            tile_position=(64, 64))

            o_t = outp.tile([128, PPG * P], bf16, tag="o")
            nc.vector.tensor_add(out=o_t[:], in0=pO[:], in1=bias_t[:])
            nc.scalar.dma_start(out=out[g], in_=o_t[:])

    nc.compile()
    return nc


def _host_indices(src_indices, tgt_perm, relationships):
    """pos_sub, pos_obj: [L, B, R] int64 — matched-list position per relation."""
    tgt = np.asarray(tgt_perm, dtype=np.int64)
    rel = np.asarray(relationships, dtype=np.int64)

    # lookup[l, b, tgt[l, b, k]] = k
    lookup = np.empty((L, B, M), dtype=np.int64)
    li = np.arange(L)[:, None, None]
    bi = np.arange(B)[None, :, None]
    lookup[li, bi, tgt] = np.broadcast_to(np.arange(M), (L, B, M))

    sub_t = np.broadcast_to(rel[None, :, :, 0], (L, B, R))
    obj_t = np.broadcast_to(rel[None, :, :, 1], (L, B, R))
    pos_sub = np.take_along_axis(lookup, sub_t, axis=2)
    pos_obj = np.take_along_axis(lookup, obj_t, axis=2)
    return pos_sub, pos_obj


def _host_prepare(hs, src_indices, tgt_perm, relationships, W_pred, b_pred):
    """Build per-core input maps."""
    import ml_dtypes
    bf16 = ml_dtypes.bfloat16

    hs = np.asarray(hs, dtype=np.float32)
    src = np.asarray(src_indices, dtype=np.int64)
    W = np.asarray(W_pred, dtype=np.float32)
    b = np.asarray(b_pred, dtype=np.float32)

    pos_sub, pos_obj = _host_indices(src_indices, tgt_perm, relationships)

    # Wcat[p, c, :] = [W_s^T | W_o^T] rows c*128+p
    WT = W.T                                                  # [2D, P]
    wcat = np.empty((2, 128, P2), dtype=np.float32)
    for c in range(2):
        wcat[c, :, 0:P] = WT[c * 128:(c + 1) * 128]
        wcat[c, :, P:P2] = WT[D + c * 128:D + (c + 1) * 128]
    wcat = np.ascontiguousarray(wcat.transpose(1, 0, 2)).astype(bf16)
    bias_b = np.ascontiguousarray(np.tile(b[None, :], (128, PPG)))
    bias_b = bias_b.astype(np.float32)

    hs_bf = hs.astype(bf16)                                   # [L, B, Q1, D]

    in_maps = []
    for cc in range(NCORES):
        sl = slice(cc * BLOC, (cc + 1) * BLOC)
        hs_core = np.ascontiguousarray(hs_bf[:, sl]).reshape(NB * Q1, D)
        src_core = np.ascontiguousarray(src[:, sl]).reshape(NB, M)
        ps_core = np.ascontiguousarray(pos_sub[:, sl]).reshape(NB, R)
        po_core = np.ascontiguousarray(pos_obj[:, sl]).reshape(NB, R)

        gidx = np.arange(NB, dtype=np.int64)[:, None] * Q1 + src_core
        gw = gidx.reshape(NGRP, NIDX // 16, 16).astype(np.int16)
        gw = np.ascontiguousarray(gw.transpose(0, 2, 1))
        gw = np.tile(gw, (1, 8, 1))                            # [NGRP,128,48]

        # position one-hots: oh[blk, k, c] with c<64 sub (r=c), c>=64 obj
        oh = np.zeros((NB, M, 2 * R), dtype=np.uint8)
        bi = np.arange(NB)[:, None]
        ri = np.arange(R)[None, :]
        oh[bi, ps_core, ri] = 1
        oh[bi, po_core, R + ri] = 1
        # pack pairs: partition = (blk%2)*64 + k, col = pk*128 + c
        ohp = oh.reshape(NGRP, PPG, 2, M, 2 * R).transpose(0, 2, 3, 1, 4)
        ohp = np.ascontiguousarray(ohp).reshape(NGRP, 128, PPG * 2 * R)

        in_maps.append({
            "hs": hs_core,
            "midx": gw,
            "ohp": ohp,
            "wcat": wcat,
            "bias": bias_b,
        })
    return in_maps


def kernel(hs, src_indices, tgt_perm, relationships, W_pred, b_pred):
    if "concourse" not in sys.modules:
        try:
            import concourse  # noqa: F401
        except ImportError:
            sys.path.insert(0, "/opt/trn_rl_repo")

    from concourse import bass_utils

    in_maps = _host_prepare(hs, src_indices, tgt_perm, relationships,
                            W_pred, b_pred)
    if "nc" not in _CACHE:
        _CACHE["nc"] = _build_program()
    nc = _CACHE["nc"]

    res = bass_utils.run_bass_kernel_spmd(nc, in_maps, list(range(NCORES)))
    outs = []
    for cc in range(NCORES):
        o = np.asarray(res.results[cc]["out"]).astype(np.float32)
        o = o.reshape(NGRP, 2, R, PPG, P).transpose(0, 3, 1, 2, 4)
        outs.append(o.reshape(L, BLOC, R, P))
    return np.concatenate(outs, axis=1)
